# revision 1
# baseline (speedup 1.0000x reference)
"""GCNII forward on 8 TRN2 NeuronCores (self-contained).

Strategy (1D row partitioning per sharding hint):
- nodes sharded 2500/core (padded 2560); edges assigned to the core owning dst.
- per layer: ht = dinv*h exchanged as fp16 via one 8-rank AllGather into a
  pair-SHARED DRAM table [20480,1024]; each core indirect-DMA-gathers its
  edges' source rows (128 rows/instr), scatter-adds them into per-dst-tile
  PSUM via one-hot fp16 matmuls, then computes z = 0.9*dinv*agg + 0.1*h0 and
  the layer GEMM z @ ((1-b)I + b*W) in fp32r (TF32-like) with the identity
  residual folded into the weights on the host.
- self-loops are real edges; gcn_norm folded into per-node dinv scaling.
"""
import math
import numpy as np

import concourse.bass as bass
import concourse.mybir as mybir
import concourse.tile as tile
from concourse import bacc
from concourse.bass_utils import run_bass_kernel_spmd
from concourse.masks import make_identity

# problem constants (hardcoded per contract)
N, E = 20000, 320000
F_IN, H, C, L = 512, 1024, 64, 8
ALPHA, THETA = 0.1, 0.5
NCORES = 8
SH = N // NCORES          # 2500 real rows per core
SHP = 2560                # padded rows per core (20*128)
V = NCORES * SHP          # padded table rows
P = 128
NT = SHP // P             # 20 dst tiles per core
KF = F_IN // P            # 4 k-tiles for W1
KH = H // P               # 8 k-tiles for H

f32 = mybir.dt.float32
f32r = mybir.dt.float32r
f16 = mybir.dt.float16
i32 = mybir.dt.int32

_cache = {}


def _preprocess(x, edge_index, W1, b1, Wg, W2, b2):
    src = np.asarray(edge_index[0], dtype=np.int64)
    dst = np.asarray(edge_index[1], dtype=np.int64)
    # self loops
    loops = np.arange(N, dtype=np.int64)
    src = np.concatenate([src, loops])
    dst = np.concatenate([dst, loops])
    deg = np.bincount(dst, minlength=N).astype(np.float32)
    dinv = 1.0 / np.sqrt(np.maximum(deg, 1.0))

    core = dst // SH
    d_loc = dst - core * SH
    tl = d_loc // P
    slot = d_loc % P
    gid = core * NT + tl
    order = np.argsort(gid, kind="stable")
    gid_s = gid[order]
    src_s = src[order]
    slot_s = slot[order]
    # rank within group
    counts = np.bincount(gid_s, minlength=NCORES * NT)
    starts = np.concatenate([[0], np.cumsum(counts)[:-1]])
    j = np.arange(len(gid_s)) - starts[gid_s]
    nchunk = int(math.ceil(counts.max() / P))
    c_idx = j // P
    p_idx = j % P
    s_tab = ((src_s // SH) * SHP + (src_s % SH)).astype(np.int32)

    offs = np.zeros((NCORES, P, NT * nchunk), dtype=np.int32)
    S = np.zeros((NCORES, NT, P, nchunk, P), dtype=np.float16)  # [core,t,e,c,d]
    core_s = gid_s // NT
    tl_s = gid_s % NT
    offs[core_s, p_idx, tl_s * nchunk + c_idx] = s_tab
    S[core_s, tl_s, p_idx, c_idx, slot_s] = np.float16(1.0)
    S = S.reshape(NCORES, NT, P, nchunk * P)

    # per-core dinv columns [P, NT]
    dinv_pad = np.zeros(NCORES * SHP, dtype=np.float32)
    idx = np.arange(N)
    dinv_pad[(idx // SH) * SHP + (idx % SH)] = dinv
    dinvc = dinv_pad.reshape(NCORES, NT, P).transpose(0, 2, 1).copy()  # [c,P,NT]
    dinv09c = (0.9 * dinvc).astype(np.float32)

    # xT shards [F_IN, SHP] padded
    x = np.asarray(x, dtype=np.float32)
    xT = np.zeros((NCORES, F_IN, SHP), dtype=np.float32)
    for c in range(NCORES):
        xT[c, :, :SH] = x[c * SH:(c + 1) * SH].T

    betas = np.log(THETA / np.arange(1.0, L + 1.0, dtype=np.float64) + 1.0)
    Wg = np.asarray(Wg, dtype=np.float64)
    eye = np.eye(H, dtype=np.float64)
    Wt = np.stack([(1.0 - betas[l]) * eye + betas[l] * Wg[l] for l in range(L)])
    Wt = Wt.astype(np.float32)

    b1b = np.broadcast_to(np.asarray(b1, np.float32), (P, H)).copy()
    b2b = np.broadcast_to(np.asarray(b2, np.float32), (P, C)).copy()

    in_maps = []
    for c in range(NCORES):
        in_maps.append({
            "xT": xT[c],
            "W1": np.asarray(W1, np.float32),
            "Wt": Wt,
            "W2": np.asarray(W2, np.float32),
            "b1b": b1b,
            "b2b": b2b,
            "dinvc": dinvc[c],
            "dinv09c": dinv09c[c],
            "offs": offs[c],
            "Smat": S[c],
        })
    return in_maps, nchunk


def _build(nchunk):
    nc = bacc.Bacc("TRN2", target_bir_lowering=False, debug=False,
                   num_devices=NCORES)
    t_xT = nc.dram_tensor("xT", [F_IN, SHP], f32r, kind="ExternalInput")
    t_W1 = nc.dram_tensor("W1", [F_IN, H], f32r, kind="ExternalInput")
    t_Wt = nc.dram_tensor("Wt", [L, H, H], f32r, kind="ExternalInput")
    t_W2 = nc.dram_tensor("W2", [H, C], f32r, kind="ExternalInput")
    t_b1 = nc.dram_tensor("b1b", [P, H], f32, kind="ExternalInput")
    t_b2 = nc.dram_tensor("b2b", [P, C], f32, kind="ExternalInput")
    t_dinv = nc.dram_tensor("dinvc", [P, NT], f32, kind="ExternalInput")
    t_dinv09 = nc.dram_tensor("dinv09c", [P, NT], f32, kind="ExternalInput")
    t_offs = nc.dram_tensor("offs", [P, NT * nchunk], i32, kind="ExternalInput")
    t_S = nc.dram_tensor("Smat", [NT, P, nchunk * P], f16, kind="ExternalInput")
    t_out = nc.dram_tensor("out", [SHP, C], f32, kind="ExternalOutput")

    h0s_dram = nc.dram_tensor("h0s", [SHP, H], f32)
    exch_in = nc.dram_tensor("exch", [SHP, H], f16)
    tables = [nc.dram_tensor(f"tbl{i}", [V, H], f16, addr_space="Shared")
              for i in range(2)]

    with tile.TileContext(nc) as tc:
        with (
            tc.tile_pool(name="const", bufs=1) as cp,
            tc.tile_pool(name="wpool", bufs=1) as wp,
            tc.tile_pool(name="spool", bufs=2) as sp,
            tc.tile_pool(name="gpool", bufs=6) as gp,
            tc.tile_pool(name="zpool", bufs=2) as zp,
            tc.tile_pool(name="ps_agg", bufs=2, space="PSUM") as pa,
            tc.tile_pool(name="ps_gemm", bufs=1, space="PSUM") as pg,
            tc.tile_pool(name="ps_tr", bufs=2, space="PSUM") as pt,
        ):
            ident = cp.tile([P, P], f32, tag="ident")
            make_identity(nc, ident[:])
            offs_sb = cp.tile([P, NT * nchunk], i32, tag="offs")
            nc.sync.dma_start(out=offs_sb[:], in_=t_offs[:])
            dinv_sb = cp.tile([P, NT], f32, tag="dinv")
            nc.sync.dma_start(out=dinv_sb[:], in_=t_dinv[:])
            dinv09_sb = cp.tile([P, NT], f32, tag="dinv09")
            nc.sync.dma_start(out=dinv09_sb[:], in_=t_dinv09[:])
            b1_sb = cp.tile([P, H], f32, tag="b1")
            nc.sync.dma_start(out=b1_sb[:], in_=t_b1[:])
            b2_sb = cp.tile([P, C], f32, tag="b2")
            nc.sync.dma_start(out=b2_sb[:], in_=t_b2[:])

            # ---- phase 0: h0 = relu(x@W1 + b1); h0s = 0.1*h0; table0 = f16(dinv*h0)
            xT_sb = cp.tile([P, KF * SHP], f32r, tag="xT")
            for k in range(KF):
                nc.sync.dma_start(out=xT_sb[:, k * SHP:(k + 1) * SHP],
                                  in_=t_xT[k * P:(k + 1) * P, :])
            W_sb = wp.tile([P, KF * H], f32r, tag="W")
            for k in range(KF):
                nc.sync.dma_start(out=W_sb[:, k * H:(k + 1) * H],
                                  in_=t_W1[k * P:(k + 1) * P, :])
            for t in range(NT):
                ps = pg.tile([P, H], f32, space="PSUM", tag="gemm")
                for k in range(KF):
                    for nh in range(2):
                        nc.tensor.matmul(
                            out=ps[:, nh * 512:(nh + 1) * 512],
                            lhsT=xT_sb[:, k * SHP + t * P: k * SHP + (t + 1) * P],
                            rhs=W_sb[:, k * H + nh * 512: k * H + (nh + 1) * 512],
                            start=(k == 0), stop=(k == KF - 1))
                nc.vector.tensor_add(out=ps[:], in0=ps[:], in1=b1_sb[:])
                h0s_t = zp.tile([P, H], f32, tag="h0w")
                nc.scalar.activation(out=h0s_t[:], in_=ps[:],
                                     func=mybir.ActivationFunctionType.Relu,
                                     scale=0.1)
                nc.sync.dma_start(out=h0s_dram[t * P:(t + 1) * P, :], in_=h0s_t[:])
                ex_t = zp.tile([P, H], f16, tag="ex")
                nc.scalar.activation(out=ex_t[:], in_=ps[:],
                                     func=mybir.ActivationFunctionType.Relu,
                                     scale=dinv_sb[:, t:t + 1])
                nc.sync.dma_start(out=exch_in[t * P:(t + 1) * P, :], in_=ex_t[:])
            nc.gpsimd.collective_compute(
                "AllGather", mybir.AluOpType.bypass,
                replica_groups=[list(range(NCORES))],
                ins=[exch_in.ap().opt()], outs=[tables[0].ap().opt()])

            W2_sb = cp.tile([P, KH * C], f32r, tag="W2")
            for k in range(KH):
                nc.sync.dma_start(out=W2_sb[:, k * C:(k + 1) * C],
                                  in_=t_W2[k * P:(k + 1) * P, :])

            # ---- layers
            for l in range(L):
                tbl = tables[l % 2]
                W_sb = wp.tile([P, KH * H], f32r, tag="W")
                for k in range(KH):
                    nc.sync.dma_start(out=W_sb[:, k * H:(k + 1) * H],
                                      in_=t_Wt[l, k * P:(k + 1) * P, :])
                for t in range(NT):
                    S_sb = sp.tile([P, nchunk * P], f16, tag="S")
                    nc.sync.dma_start(out=S_sb[:], in_=t_S[t])
                    agg = pa.tile([P, H], f32, space="PSUM", tag="agg")
                    for c in range(nchunk):
                        g_sb = gp.tile([P, H], f16, tag="g")
                        nc.gpsimd.indirect_dma_start(
                            out=g_sb[:], out_offset=None, in_=tbl.ap(),
                            in_offset=bass.IndirectOffsetOnAxis(
                                ap=offs_sb[:, t * nchunk + c: t * nchunk + c + 1],
                                axis=0))
                        for nh in range(2):
                            nc.tensor.matmul(
                                out=agg[:, nh * 512:(nh + 1) * 512],
                                lhsT=S_sb[:, c * P:(c + 1) * P],
                                rhs=g_sb[:, nh * 512:(nh + 1) * 512],
                                start=(c == 0), stop=(c == nchunk - 1))
                    # z = 0.9*dinv*agg + 0.1*h0   (as f32r for the GEMM)
                    h0s_t = zp.tile([P, H], f32, tag="h0r")
                    nc.sync.dma_start(out=h0s_t[:],
                                      in_=h0s_dram[t * P:(t + 1) * P, :])
                    z0 = zp.tile([P, H], f32, tag="z0")
                    nc.vector.tensor_scalar(
                        out=z0[:], in0=agg[:], scalar1=dinv09_sb[:, t:t + 1],
                        scalar2=None, op0=mybir.AluOpType.mult)
                    z = zp.tile([P, H], f32, tag="z")
                    nc.vector.tensor_add(out=z[:], in0=z0[:], in1=h0s_t[:])
                    # transpose z -> zT (8 k-tiles)
                    zT = zp.tile([P, KH * P], f32r, tag="zT")
                    for k in range(KH):
                        trp = pt.tile([P, P], f32, space="PSUM", tag="tr")
                        nc.tensor.transpose(out=trp[:],
                                            in_=z[:, k * P:(k + 1) * P],
                                            identity=ident[:])
                        nc.vector.tensor_copy(out=zT[:, k * P:(k + 1) * P],
                                              in_=trp[:])
                    ps = pg.tile([P, H], f32, space="PSUM", tag="gemm")
                    for k in range(KH):
                        for nh in range(2):
                            nc.tensor.matmul(
                                out=ps[:, nh * 512:(nh + 1) * 512],
                                lhsT=zT[:, k * P:(k + 1) * P],
                                rhs=W_sb[:, k * H + nh * 512: k * H + (nh + 1) * 512],
                                start=(k == 0), stop=(k == KH - 1))
                    if l < L - 1:
                        ex_t = zp.tile([P, H], f16, tag="ex")
                        nc.scalar.activation(out=ex_t[:], in_=ps[:],
                                             func=mybir.ActivationFunctionType.Relu,
                                             scale=dinv_sb[:, t:t + 1])
                        nc.sync.dma_start(out=exch_in[t * P:(t + 1) * P, :],
                                          in_=ex_t[:])
                    else:
                        # h8 tile (f32r) -> logits -> log_softmax -> out
                        h8 = zp.tile([P, H], f32, tag="z")
                        nc.scalar.activation(out=h8[:], in_=ps[:],
                                             func=mybir.ActivationFunctionType.Relu)
                        hT = zp.tile([P, KH * P], f32r, tag="zT")
                        for k in range(KH):
                            trp = pt.tile([P, P], f32, space="PSUM", tag="tr")
                            nc.tensor.transpose(out=trp[:],
                                                in_=h8[:, k * P:(k + 1) * P],
                                                identity=ident[:])
                            nc.vector.tensor_copy(out=hT[:, k * P:(k + 1) * P],
                                                  in_=trp[:])
                        psl = pt.tile([P, C], f32, space="PSUM", tag="tr")
                        for k in range(KH):
                            nc.tensor.matmul(
                                out=psl[:],
                                lhsT=hT[:, k * P:(k + 1) * P],
                                rhs=W2_sb[:, k * C:(k + 1) * C],
                                start=(k == 0), stop=(k == KH - 1))
                        nc.vector.tensor_add(out=psl[:], in0=psl[:], in1=b2_sb[:])
                        mx = zp.tile([P, 1], f32, tag="mx")
                        nc.vector.tensor_reduce(out=mx[:], in_=psl[:],
                                                axis=mybir.AxisListType.X,
                                                op=mybir.AluOpType.max)
                        nmx = zp.tile([P, 1], f32, tag="nmx")
                        nc.vector.tensor_scalar(
                            out=nmx[:], in0=mx[:], scalar1=-1.0, scalar2=None,
                            op0=mybir.AluOpType.mult)
                        esb = zp.tile([P, C], f32, tag="esb")
                        se = zp.tile([P, 1], f32, tag="se")
                        nc.scalar.activation(out=esb[:], in_=psl[:],
                                             func=mybir.ActivationFunctionType.Exp,
                                             bias=nmx[:], accum_out=se[:])
                        lse = zp.tile([P, 1], f32, tag="lse")
                        nc.scalar.activation(out=lse[:], in_=se[:],
                                             func=mybir.ActivationFunctionType.Ln)
                        o_t = zp.tile([P, C], f32, tag="ot")
                        nc.vector.tensor_scalar(
                            out=o_t[:], in0=psl[:], scalar1=mx[:], scalar2=lse[:],
                            op0=mybir.AluOpType.subtract,
                            op1=mybir.AluOpType.subtract)
                        nc.sync.dma_start(out=t_out[t * P:(t + 1) * P, :],
                                          in_=o_t[:])
                if l < L - 1:
                    nc.gpsimd.collective_compute(
                        "AllGather", mybir.AluOpType.bypass,
                        replica_groups=[list(range(NCORES))],
                        ins=[exch_in.ap().opt()],
                        outs=[tables[(l + 1) % 2].ap().opt()])
    nc.compile()
    return nc


def kernel(**inputs):
    in_maps, nchunk = _preprocess(
        inputs["x"], inputs["edge_index"], inputs["W1"], inputs["b1"],
        inputs["Wg"], inputs["W2"], inputs["b2"])
    key = ("nc", nchunk)
    if key not in _cache:
        _cache[key] = _build(nchunk)
    nc = _cache[key]
    res = run_bass_kernel_spmd(nc, in_maps, list(range(NCORES)))
    out = np.concatenate(
        [res.results[c]["out"][:SH] for c in range(NCORES)], axis=0)
    return out.astype(np.float32)



# revision 7
# speedup vs baseline: 1.2772x; 1.2772x over previous
"""GCNII forward on 8 TRN2 NeuronCores (self-contained).

Strategy (1D row partitioning):
- nodes sharded 2500/core (padded 2560); edges assigned to the core owning dst.
- exchange table in fp8e4 (dinv*h), AllGathered in 2 half-shard slices for
  partial overlap with the tile loop; double-buffered across layers.
- per dst tile: ONE batched dma_gather pulls all ~2300 source rows into
  [128, nch, 1024] fp8 SBUF; scatter-add via one-hot fp8 DoubleRow matmuls
  (2 edge-chunks per instruction).
- layer GEMM in fp8 DoubleRow with the identity-residual path kept in f32:
  h = relu((1-b)*z + (b/s)*(q8(z) @ q8(s*b*Wg))), s a power of two.
- h0 residual (0.1*h0) kept in SBUF as bf16; phase0 GEMM in bf16.
"""
import numpy as np
from ml_dtypes import float8_e4m3, bfloat16

import concourse.bass as bass
import concourse.mybir as mybir
import concourse.tile as tile
from concourse import bacc
from concourse.bass_utils import run_bass_kernel_spmd
from concourse.masks import make_identity

N, E = 20000, 320000
F_IN, H, C, L = 512, 1024, 64, 8
ALPHA, THETA = 0.1, 0.5
NCORES = 8
SH = N // NCORES            # 2500 real rows per core
SHP = 2560                  # padded rows per core (20*128)
HALF = SHP // 2             # AllGather half-shard
V = NCORES * SHP            # table rows
P = 128
NT = SHP // P               # 20 dst tiles per core
KF = F_IN // P              # 4 k-tiles for W1
KH = H // P                 # 8 k-tiles for H

f32 = mybir.dt.float32
bf16 = mybir.dt.bfloat16
fp8 = mybir.dt.float8e4
i16 = mybir.dt.int16

BETAS = np.log(THETA / np.arange(1.0, L + 1.0) + 1.0).astype(np.float64)

_cache = {}


def _table_row(node):
    """Row in the AllGathered table (2-half layout) for global node ids."""
    o = node // SH
    i = node - o * SH
    return np.where(i < HALF, o * HALF + i,
                    NCORES * HALF + o * HALF + (i - HALF)).astype(np.int64)


def _preprocess(x, edge_index, W1, b1, Wg, W2, b2):
    src = np.asarray(edge_index[0], dtype=np.int64)
    dst = np.asarray(edge_index[1], dtype=np.int64)
    loops = np.arange(N, dtype=np.int64)
    src = np.concatenate([src, loops])
    dst = np.concatenate([dst, loops])
    deg = np.bincount(dst, minlength=N).astype(np.float32)
    dinv = 1.0 / np.sqrt(np.maximum(deg, 1.0))

    core = dst // SH
    d_loc = dst - core * SH
    tl = d_loc // P
    slot = d_loc % P
    gid = core * NT + tl
    order = np.argsort(gid, kind="stable")
    gid_s = gid[order]
    rows_s = _table_row(src[order])
    slot_s = slot[order]
    counts = np.bincount(gid_s, minlength=NCORES * NT)
    starts = np.concatenate([[0], np.cumsum(counts)[:-1]])
    j = np.arange(len(gid_s)) - starts[gid_s]
    # per-tile chunk counts: max over cores (SPMD shares one program),
    # padded to even for DoubleRow pairs
    nch_t = np.ceil(counts.reshape(NCORES, NT) / P).astype(np.int64).max(0)
    nch_t += nch_t % 2
    nch_t = np.maximum(nch_t, 2)                # [NT], same for all cores
    TOTC = int(nch_t.sum())
    base = np.zeros(NT, dtype=np.int64)
    base[1:] = np.cumsum(nch_t)[:-1]

    c_idx = j // P
    p_idx = j % P

    idx16 = np.zeros((NCORES, 128, TOTC * 8), dtype=np.int16)
    S = np.zeros((NCORES, P, TOTC, P), dtype=float8_e4m3)
    core_s = gid_s // NT
    tl_s = gid_s % NT
    gcol = base[tl_s] + c_idx                   # chunk column within core
    flat = gcol * P + p_idx                     # flat idx position per core
    # idx wrap: element i -> (partition i%16 [replicated *8], col i//16)
    icol = flat // 16
    irow = flat % 16
    for r in range(8):
        idx16[core_s, r * 16 + irow, icol] = rows_s.astype(np.int16)
    S[core_s, p_idx, gcol, slot_s] = 1.0

    # per-core padded dinv columns [P, NT] (0 on pad slots)
    dpad = np.zeros(NCORES * SHP, dtype=np.float32)
    idx = np.arange(N)
    dpad[(idx // SH) * SHP + (idx % SH)] = dinv
    dcols = dpad.reshape(NCORES, NT, P).transpose(0, 2, 1).copy()  # [c,P,NT]
    d09 = (0.9 * dcols).astype(np.float32)
    escale = np.zeros((NCORES, P, L * NT), dtype=np.float32)
    for l in range(L):
        escale[:, :, l * NT:(l + 1) * NT] = (1.0 - BETAS[l]) * dcols

    # x packed for lhsT: xpack[t, p, k*128+m] = x[t*128+m, k*128+p]
    x = np.asarray(x, dtype=np.float32)
    xp = np.zeros((NCORES, NT, P, F_IN), dtype=bfloat16)
    for c in range(NCORES):
        xc = np.zeros((SHP, F_IN), np.float32)
        xc[:SH] = x[c * SH:(c + 1) * SH]
        # [NT,P(m),F_IN(kp)] -> transpose to [NT, kp, m] -> view [NT,KF,P,P]
        xt = xc.reshape(NT, P, KF, P).transpose(0, 3, 2, 1)  # [t, p, k, m]
        xp[c] = xt.reshape(NT, P, F_IN).astype(bfloat16)

    def pack_w(w, dt):  # [K, Nout] -> [P, K//P, Nout]
        K = w.shape[0]
        return np.ascontiguousarray(
            w.reshape(K // P, P, -1).transpose(1, 0, 2)).astype(dt)

    W1p = pack_w(np.asarray(W1, np.float32), bfloat16)          # [P,KF,H]
    W2p = pack_w(np.asarray(W2, np.float32), bfloat16)          # [P,KH,C]
    Wg = np.asarray(Wg, dtype=np.float64)
    scales = []
    Wqs = []
    for l in range(L):
        bw = BETAS[l] * Wg[l]
        s = 2.0 ** np.floor(np.log2(240.0 / np.abs(bw).max()))
        scales.append(float(s))
        Wqs.append(pack_w((s * bw).astype(np.float32), float8_e4m3))
    Wq = np.stack(Wqs)                                          # [L,P,KH,H]

    b1b = np.broadcast_to(np.asarray(b1, np.float32), (P, H)).copy()
    b2b = np.broadcast_to(np.asarray(b2, np.float32), (P, C)).copy()

    meta = {"nch": nch_t, "base": base, "TOTC": TOTC, "scales": scales}
    in_maps = []
    for c in range(NCORES):
        in_maps.append({
            "xp": xp[c],
            "W1p": W1p, "W2p": W2p, "Wq": Wq,
            "b1b": b1b, "b2b": b2b,
            "d09": d09[c], "dinvc": dcols[c].astype(np.float32),
            "escale": escale[c],
            "idx16": idx16[c], "Smat": S[c],
        })
    return in_maps, meta


def _build(meta):
    TOTC = meta["TOTC"]
    nch0 = meta["nch"]
    base0 = meta["base"]
    scales = meta["scales"]
    NCHMAX = int(nch0.max())

    nc = bacc.Bacc("TRN2", target_bir_lowering=False, debug=False,
                   num_devices=NCORES)
    t_xp = nc.dram_tensor("xp", [NT, P, F_IN], bf16, kind="ExternalInput")
    t_W1 = nc.dram_tensor("W1p", [P, KF, H], bf16, kind="ExternalInput")
    t_W2 = nc.dram_tensor("W2p", [P, KH, C], bf16, kind="ExternalInput")
    t_Wq = nc.dram_tensor("Wq", [L, P, KH, H], fp8, kind="ExternalInput")
    t_b1 = nc.dram_tensor("b1b", [P, H], f32, kind="ExternalInput")
    t_b2 = nc.dram_tensor("b2b", [P, C], f32, kind="ExternalInput")
    t_d09 = nc.dram_tensor("d09", [P, NT], f32, kind="ExternalInput")
    t_dinv = nc.dram_tensor("dinvc", [P, NT], f32, kind="ExternalInput")
    t_esc = nc.dram_tensor("escale", [P, L * NT], f32, kind="ExternalInput")
    t_idx = nc.dram_tensor("idx16", [128, TOTC * 8], i16, kind="ExternalInput")
    t_S = nc.dram_tensor("Smat", [P, TOTC, P], fp8, kind="ExternalInput")
    t_out = nc.dram_tensor("out", [SHP, C], f32, kind="ExternalOutput")

    exch = nc.dram_tensor("exch", [SHP, H], fp8)
    tables = [nc.dram_tensor(f"tbl{i}", [V, H], fp8, addr_space="Shared")
              for i in range(2)]

    DR = mybir.MatmulPerfMode.DoubleRow
    ACT = mybir.ActivationFunctionType

    def allgather_halves(dst_tbl):
        for hf in range(2):
            nc.gpsimd.collective_compute(
                "AllGather", mybir.AluOpType.bypass,
                replica_groups=[list(range(NCORES))],
                ins=[exch.ap()[hf * HALF:(hf + 1) * HALF].opt()],
                outs=[dst_tbl.ap()[hf * NCORES * HALF:
                                   (hf + 1) * NCORES * HALF].opt()])

    with tile.TileContext(nc) as tc:
        with (
            tc.tile_pool(name="const", bufs=1) as cp,
            tc.tile_pool(name="wpool", bufs=2) as wp,
            tc.tile_pool(name="xpool", bufs=2) as xp_,
            tc.tile_pool(name="gpool", bufs=2) as gp,
            tc.tile_pool(name="zpool", bufs=2) as zp,
            tc.tile_pool(name="ps_agg", bufs=2, space="PSUM") as pa,
            tc.tile_pool(name="ps_gemm", bufs=1, space="PSUM") as pg,
            tc.tile_pool(name="ps_tr", bufs=2, space="PSUM") as pt,
        ):
            ident = cp.tile([P, P], f32, tag="ident")
            make_identity(nc, ident[:])
            idx_sb = cp.tile([128, TOTC * 8], i16, tag="idx")
            nc.sync.dma_start(out=idx_sb[:], in_=t_idx[:])
            S_sb = cp.tile([P, TOTC, P], fp8, tag="S")
            nc.sync.dma_start(out=S_sb[:], in_=t_S[:])
            d09_sb = cp.tile([P, NT], f32, tag="d09")
            nc.sync.dma_start(out=d09_sb[:], in_=t_d09[:])
            dinv_sb = cp.tile([P, NT], f32, tag="dinv")
            nc.sync.dma_start(out=dinv_sb[:], in_=t_dinv[:])
            esc_sb = cp.tile([P, L * NT], f32, tag="esc")
            nc.sync.dma_start(out=esc_sb[:], in_=t_esc[:])
            b1_sb = cp.tile([P, H], f32, tag="b1")
            nc.sync.dma_start(out=b1_sb[:], in_=t_b1[:])
            b2_sb = cp.tile([P, C], f32, tag="b2")
            nc.sync.dma_start(out=b2_sb[:], in_=t_b2[:])
            W1_sb = cp.tile([P, KF, H], bf16, tag="W1")
            nc.sync.dma_start(out=W1_sb[:], in_=t_W1[:])
            W2_sb = cp.tile([P, KH, C], bf16, tag="W2")
            nc.sync.dma_start(out=W2_sb[:], in_=t_W2[:])
            h0s_sb = cp.tile([P, NT, H], bf16, tag="h0s")

            # ---- phase 0: h0 = relu(x@W1 + b1); h0s = bf16(0.1*h0);
            #      exch = fp8(dinv*h0) -> AllGather -> tables[0]
            for t in range(NT):
                xt = xp_.tile([P, KF, P], bf16, tag="xt")
                nc.sync.dma_start(out=xt[:], in_=t_xp[t])
                ps = pg.tile([P, H], f32, space="PSUM", tag="gemm")
                for k in range(KF):
                    for nh in range(2):
                        nc.tensor.matmul(
                            out=ps[:, nh * 512:(nh + 1) * 512],
                            lhsT=xt[:, k, :],
                            rhs=W1_sb[:, k, nh * 512:(nh + 1) * 512],
                            start=(k == 0), stop=(k == KF - 1))
                nc.vector.tensor_add(out=ps[:], in0=ps[:], in1=b1_sb[:])
                nc.scalar.activation(out=h0s_sb[:, t, :], in_=ps[:],
                                     func=ACT.Relu, scale=0.1)
                e0 = zp.tile([P, H], fp8, tag="ex")
                nc.scalar.activation(out=e0[:], in_=ps[:], func=ACT.Relu,
                                     scale=dinv_sb[:, t:t + 1])
                nc.sync.dma_start(out=exch[t * P:(t + 1) * P, :], in_=e0[:])
            allgather_halves(tables[0])

            # ---- layers
            for l in range(L):
                tbl = tables[l % 2]
                beta = float(BETAS[l])
                cprime = beta / ((1.0 - beta) * scales[l])
                Wq_sb = wp.tile([P, KH, H], fp8, tag="W")
                nc.sync.dma_start(out=Wq_sb[:], in_=t_Wq[l])
                for t in range(NT):
                    nch_t = int(nch0[t])
                    b8 = int(base0[t]) * 8
                    bS = int(base0[t])
                    g = gp.tile([P, NCHMAX, H], fp8, tag="g")
                    for cc0 in range(0, nch_t, 8):
                        w8 = min(8, nch_t - cc0)
                        nc.gpsimd.dma_gather(
                            out_ap=g[:, cc0:cc0 + w8, :], in_ap=tbl.ap(),
                            idxs_ap=idx_sb[:, b8 + cc0 * 8:
                                           b8 + (cc0 + w8) * 8],
                            num_idxs=w8 * P, num_idxs_reg=w8 * P,
                            elem_size=H)
                    agg = pa.tile([P, H], f32, space="PSUM", tag="agg")
                    for kp in range(nch_t // 2):
                        for nh in range(2):
                            nc.tensor.matmul(
                                out=agg[:, nh * 512:(nh + 1) * 512],
                                lhsT=S_sb[:, bS + 2 * kp:bS + 2 * kp + 2, :],
                                rhs=g[:, 2 * kp:2 * kp + 2,
                                      nh * 512:(nh + 1) * 512],
                                start=(kp == 0), stop=(kp == nch_t // 2 - 1),
                                perf_mode=DR)
                    # z = 0.9*dinv*agg + 0.1*h0
                    z = zp.tile([P, H], f32, tag="z")
                    nc.scalar.activation(out=z[:], in_=agg[:], func=ACT.Copy,
                                         scale=d09_sb[:, t:t + 1])
                    nc.vector.tensor_add(out=z[:], in0=z[:],
                                         in1=h0s_sb[:, t, :])
                    zqT = zp.tile([P, KH, P], fp8, tag="zqT")
                    for k in range(KH):
                        trp = pt.tile([P, P], f32, space="PSUM", tag="tr")
                        nc.tensor.transpose(out=trp[:],
                                            in_=z[:, k * P:(k + 1) * P],
                                            identity=ident[:])
                        nc.vector.tensor_copy(out=zqT[:, k, :], in_=trp[:])
                    ps2 = pg.tile([P, H], f32, space="PSUM", tag="gemm")
                    for kp in range(KH // 2):
                        for nh in range(2):
                            nc.tensor.matmul(
                                out=ps2[:, nh * 512:(nh + 1) * 512],
                                lhsT=zqT[:, 2 * kp:2 * kp + 2, :],
                                rhs=Wq_sb[:, 2 * kp:2 * kp + 2,
                                          nh * 512:(nh + 1) * 512],
                                start=(kp == 0), stop=(kp == KH // 2 - 1),
                                perf_mode=DR)
                    # w = z + (beta/((1-beta)*s))*ps2 ; in-place onto t3
                    w = zp.tile([P, H], f32, tag="w")
                    nc.scalar.activation(out=w[:], in_=ps2[:], func=ACT.Copy,
                                         scale=cprime)
                    nc.vector.tensor_add(out=w[:], in0=w[:], in1=z[:])
                    if l < L - 1:
                        e = zp.tile([P, H], fp8, tag="ex")
                        nc.scalar.activation(
                            out=e[:], in_=w[:], func=ACT.Relu,
                            scale=esc_sb[:, l * NT + t:l * NT + t + 1])
                        nc.sync.dma_start(out=exch[t * P:(t + 1) * P, :],
                                          in_=e[:])
                    else:
                        h8 = zp.tile([P, H], f32, tag="h8")
                        nc.scalar.activation(out=h8[:], in_=w[:],
                                             func=ACT.Relu,
                                             scale=1.0 - beta)
                        h8T = zp.tile([P, KH, P], bf16, tag="h8T")
                        for k in range(KH):
                            trp = pt.tile([P, P], f32, space="PSUM", tag="tr")
                            nc.tensor.transpose(out=trp[:],
                                                in_=h8[:, k * P:(k + 1) * P],
                                                identity=ident[:])
                            nc.vector.tensor_copy(out=h8T[:, k, :], in_=trp[:])
                        psl = pt.tile([P, P], f32, space="PSUM", tag="tr")
                        for k in range(KH):
                            nc.tensor.matmul(
                                out=psl[:, 0:C],
                                lhsT=h8T[:, k, :],
                                rhs=W2_sb[:, k, :],
                                start=(k == 0), stop=(k == KH - 1))
                        nc.vector.tensor_add(out=psl[:, 0:C], in0=psl[:, 0:C],
                                             in1=b2_sb[:])
                        mx = zp.tile([P, 1], f32, tag="mx")
                        nc.vector.tensor_reduce(out=mx[:], in_=psl[:, 0:C],
                                                axis=mybir.AxisListType.X,
                                                op=mybir.AluOpType.max)
                        nmx = zp.tile([P, 1], f32, tag="nmx")
                        nc.vector.tensor_scalar(
                            out=nmx[:], in0=mx[:], scalar1=-1.0, scalar2=None,
                            op0=mybir.AluOpType.mult)
                        esb = zp.tile([P, C], f32, tag="esb")
                        se = zp.tile([P, 1], f32, tag="se")
                        nc.scalar.activation(out=esb[:], in_=psl[:, 0:C],
                                             func=ACT.Exp,
                                             bias=nmx[:], accum_out=se[:])
                        lse = zp.tile([P, 1], f32, tag="lse")
                        nc.scalar.activation(out=lse[:], in_=se[:],
                                             func=ACT.Ln)
                        o_t = zp.tile([P, C], f32, tag="ot")
                        nc.vector.tensor_scalar(
                            out=o_t[:], in0=psl[:, 0:C], scalar1=mx[:],
                            scalar2=lse[:],
                            op0=mybir.AluOpType.subtract,
                            op1=mybir.AluOpType.subtract)
                        nc.sync.dma_start(out=t_out[t * P:(t + 1) * P, :],
                                          in_=o_t[:])
                if l < L - 1:
                    allgather_halves(tables[(l + 1) % 2])
    nc.compile()
    return nc


def kernel(**inputs):
    in_maps, meta = _preprocess(
        inputs["x"], inputs["edge_index"], inputs["W1"], inputs["b1"],
        inputs["Wg"], inputs["W2"], inputs["b2"])
    key = ("nc", meta["TOTC"], tuple(meta["nch"]), tuple(meta["scales"]))
    if key not in _cache:
        _cache[key] = _build(meta)
    nc = _cache[key]
    res = run_bass_kernel_spmd(nc, in_maps, list(range(NCORES)))
    out = np.concatenate(
        [res.results[c]["out"][:SH] for c in range(NCORES)], axis=0)
    return out.astype(np.float32)


# revision 14
# speedup vs baseline: 3.1054x; 2.4314x over previous
"""GCNII forward on 8 TRN2 NeuronCores (self-contained).

Strategy (1D row partitioning):
- nodes sharded 2500/core (padded 2560); edges assigned to the core owning dst.
- exchange table in fp8e4 (dinv*h), AllGathered in 2 half-shard slices for
  partial overlap with the tile loop; double-buffered across layers.
- per dst tile: ONE batched dma_gather pulls all ~2300 source rows into
  [128, nch, 1024] fp8 SBUF; scatter-add via one-hot fp8 DoubleRow matmuls
  (2 edge-chunks per instruction).
- layer GEMM in fp8 DoubleRow with the identity-residual path kept in f32:
  h = relu((1-b)*z + (b/s)*(q8(z) @ q8(s*b*Wg))), s a power of two.
- h0 residual (0.1*h0) kept in SBUF as bf16; phase0 GEMM in bf16.
"""
import numpy as np
from ml_dtypes import float8_e4m3, bfloat16

import concourse.bass as bass
import concourse.mybir as mybir
import concourse.tile as tile
from concourse import bacc
from concourse.bass_utils import run_bass_kernel_spmd
from concourse.masks import make_identity

N, E = 20000, 320000
F_IN, H, C, L = 512, 1024, 64, 8
ALPHA, THETA = 0.1, 0.5
NCORES = 8
SH = N // NCORES            # 2500 real rows per core
SHP = 2560                  # padded rows per core (20*128)
HALF = SHP // 2             # AllGather half-shard
V = NCORES * SHP            # table rows
P = 128
NT = SHP // P               # 20 dst tiles per core
KF = F_IN // P              # 4 k-tiles for W1
KH = H // P                 # 8 k-tiles for H

f32 = mybir.dt.float32
bf16 = mybir.dt.bfloat16
fp8 = mybir.dt.float8e4
i16 = mybir.dt.int16

BETAS = np.log(THETA / np.arange(1.0, L + 1.0) + 1.0).astype(np.float64)

_cache = {}


def _table_row(node):
    """Row in the AllGathered table (2-half layout) for global node ids."""
    o = node // SH
    i = node - o * SH
    return np.where(i < HALF, o * HALF + i,
                    NCORES * HALF + o * HALF + (i - HALF)).astype(np.int64)


def _preprocess(x, edge_index, W1, b1, Wg, W2, b2):
    src = np.asarray(edge_index[0], dtype=np.int64)
    dst = np.asarray(edge_index[1], dtype=np.int64)
    loops = np.arange(N, dtype=np.int64)
    src = np.concatenate([src, loops])
    dst = np.concatenate([dst, loops])
    deg = np.bincount(dst, minlength=N).astype(np.float32)
    dinv = 1.0 / np.sqrt(np.maximum(deg, 1.0))

    core = dst // SH
    d_loc = dst - core * SH
    tl = d_loc // P
    slot = d_loc % P
    gid = core * NT + tl
    order = np.argsort(gid, kind="stable")
    gid_s = gid[order]
    rows_s = _table_row(src[order])
    slot_s = slot[order]
    counts = np.bincount(gid_s, minlength=NCORES * NT)
    starts = np.concatenate([[0], np.cumsum(counts)[:-1]])
    j = np.arange(len(gid_s)) - starts[gid_s]
    # per-tile chunk counts: max over cores (SPMD shares one program),
    # padded to even for DoubleRow pairs
    nch_t = np.ceil(counts.reshape(NCORES, NT) / P).astype(np.int64).max(0)
    nch_t += nch_t % 2
    nch_t = np.maximum(nch_t, 2)                # [NT], same for all cores
    TOTC = int(nch_t.sum())
    base = np.zeros(NT, dtype=np.int64)
    base[1:] = np.cumsum(nch_t)[:-1]

    c_idx = j // P
    p_idx = j % P

    idx16 = np.zeros((NCORES, 128, TOTC * 8), dtype=np.int16)
    S = np.zeros((NCORES, P, TOTC, P), dtype=float8_e4m3)
    core_s = gid_s // NT
    tl_s = gid_s % NT
    gcol = base[tl_s] + c_idx                   # chunk column within core
    flat = gcol * P + p_idx                     # flat idx position per core
    # idx wrap: element i -> (partition i%16 [replicated *8], col i//16)
    icol = flat // 16
    irow = flat % 16
    for r in range(8):
        idx16[core_s, r * 16 + irow, icol] = rows_s.astype(np.int16)
    S[core_s, p_idx, gcol, slot_s] = 1.0

    # per-core padded dinv columns [P, NT] (0 on pad slots)
    dpad = np.zeros(NCORES * SHP, dtype=np.float32)
    idx = np.arange(N)
    dpad[(idx // SH) * SHP + (idx % SH)] = dinv
    dcols = dpad.reshape(NCORES, NT, P).transpose(0, 2, 1).copy()  # [c,P,NT]
    d09 = (0.9 * dcols).astype(np.float32)
    escale = np.zeros((NCORES, P, L * NT), dtype=np.float32)
    for l in range(L):
        escale[:, :, l * NT:(l + 1) * NT] = (1.0 - BETAS[l]) * dcols

    # x packed for lhsT: xpack[t, p, k*128+m] = x[t*128+m, k*128+p]
    x = np.asarray(x, dtype=np.float32)
    xp = np.zeros((NCORES, NT, P, F_IN), dtype=bfloat16)
    for c in range(NCORES):
        xc = np.zeros((SHP, F_IN), np.float32)
        xc[:SH] = x[c * SH:(c + 1) * SH]
        # [NT,P(m),F_IN(kp)] -> transpose to [NT, kp, m] -> view [NT,KF,P,P]
        xt = xc.reshape(NT, P, KF, P).transpose(0, 3, 2, 1)  # [t, p, k, m]
        xp[c] = xt.reshape(NT, P, F_IN).astype(bfloat16)

    def pack_w(w, dt):  # [K, Nout] -> [P, K//P, Nout]
        K = w.shape[0]
        return np.ascontiguousarray(
            w.reshape(K // P, P, -1).transpose(1, 0, 2)).astype(dt)

    W1p = pack_w(np.asarray(W1, np.float32), bfloat16)          # [P,KF,H]
    W2p = pack_w(np.asarray(W2, np.float32), bfloat16)          # [P,KH,C]
    Wg = np.asarray(Wg, dtype=np.float64)
    scales = []
    Wqs = []
    for l in range(L):
        bw = BETAS[l] * Wg[l]
        s = 2.0 ** np.floor(np.log2(240.0 / np.abs(bw).max()))
        scales.append(float(s))
        Wqs.append(pack_w((s * bw).astype(np.float32), float8_e4m3))
    Wq = np.stack(Wqs)                                          # [L,P,KH,H]

    b1b = np.broadcast_to(np.asarray(b1, np.float32), (P, H)).copy()
    b2b = np.broadcast_to(np.asarray(b2, np.float32), (P, C)).copy()

    meta = {"nch": nch_t, "base": base, "TOTC": TOTC, "scales": scales}
    in_maps = []
    for c in range(NCORES):
        in_maps.append({
            "xp": xp[c],
            "W1p": W1p, "W2p": W2p, "Wq": Wq,
            "b1b": b1b, "b2b": b2b,
            "d09": d09[c], "dinvc": dcols[c].astype(np.float32),
            "escale": escale[c],
            "idx16": idx16[c], "Smat": S[c],
        })
    return in_maps, meta


def _build(meta):
    TOTC = meta["TOTC"]
    nch0 = meta["nch"]
    base0 = meta["base"]
    scales = meta["scales"]
    NCHMAX = int(nch0.max())

    nc = bacc.Bacc("TRN2", target_bir_lowering=False, debug=False,
                   num_devices=NCORES, num_swdge_queues=4)
    t_xp = nc.dram_tensor("xp", [NT, P, F_IN], bf16, kind="ExternalInput")
    t_W1 = nc.dram_tensor("W1p", [P, KF, H], bf16, kind="ExternalInput")
    t_W2 = nc.dram_tensor("W2p", [P, KH, C], bf16, kind="ExternalInput")
    t_Wq = nc.dram_tensor("Wq", [L, P, KH, H], fp8, kind="ExternalInput")
    t_b1 = nc.dram_tensor("b1b", [P, H], f32, kind="ExternalInput")
    t_b2 = nc.dram_tensor("b2b", [P, C], f32, kind="ExternalInput")
    t_d09 = nc.dram_tensor("d09", [P, NT], f32, kind="ExternalInput")
    t_dinv = nc.dram_tensor("dinvc", [P, NT], f32, kind="ExternalInput")
    t_esc = nc.dram_tensor("escale", [P, L * NT], f32, kind="ExternalInput")
    t_idx = nc.dram_tensor("idx16", [128, TOTC * 8], i16, kind="ExternalInput")
    t_S = nc.dram_tensor("Smat", [P, TOTC, P], fp8, kind="ExternalInput")
    t_out = nc.dram_tensor("out", [SHP, C], f32, kind="ExternalOutput")

    exch = nc.dram_tensor("exch", [SHP, H], fp8)
    tables = [nc.dram_tensor(f"tbl{i}", [V, H], fp8, addr_space="Shared")
              for i in range(2)]

    DR = mybir.MatmulPerfMode.DoubleRow
    ACT = mybir.ActivationFunctionType

    def allgather_half(dst_tbl, hf):
        nc.gpsimd.collective_compute(
            "AllGather", mybir.AluOpType.bypass,
            replica_groups=[list(range(NCORES))],
            ins=[exch.ap()[hf * HALF:(hf + 1) * HALF].opt()],
            outs=[dst_tbl.ap()[hf * NCORES * HALF:
                               (hf + 1) * NCORES * HALF].opt()])

    with tile.TileContext(nc) as tc:
        with (
            tc.tile_pool(name="const", bufs=1) as cp,
            tc.tile_pool(name="wpool", bufs=2) as wp,
            tc.tile_pool(name="xpool", bufs=2) as xp_,
            tc.tile_pool(name="gpool", bufs=2) as gp,
            tc.tile_pool(name="zpool", bufs=2) as zp,
            tc.tile_pool(name="ps_agg", bufs=2, space="PSUM") as pa,
            tc.tile_pool(name="ps_gemm", bufs=1, space="PSUM") as pg,
            tc.tile_pool(name="ps_tr", bufs=2, space="PSUM") as pt,
        ):
            ident = cp.tile([P, P], f32, tag="ident")
            make_identity(nc, ident[:])
            idx_sb = cp.tile([128, TOTC * 8], i16, tag="idx")
            nc.sync.dma_start(out=idx_sb[:], in_=t_idx[:])
            S_sb = cp.tile([P, TOTC, P], fp8, tag="S")
            nc.sync.dma_start(out=S_sb[:], in_=t_S[:])
            d09_sb = cp.tile([P, NT], f32, tag="d09")
            nc.sync.dma_start(out=d09_sb[:], in_=t_d09[:])
            dinv_sb = cp.tile([P, NT], f32, tag="dinv")
            nc.sync.dma_start(out=dinv_sb[:], in_=t_dinv[:])
            esc_sb = cp.tile([P, L * NT], f32, tag="esc")
            nc.sync.dma_start(out=esc_sb[:], in_=t_esc[:])
            b1_sb = cp.tile([P, H], f32, tag="b1")
            nc.sync.dma_start(out=b1_sb[:], in_=t_b1[:])
            b2_sb = cp.tile([P, C], f32, tag="b2")
            nc.sync.dma_start(out=b2_sb[:], in_=t_b2[:])
            W1_sb = cp.tile([P, KF, H], bf16, tag="W1")
            nc.sync.dma_start(out=W1_sb[:], in_=t_W1[:])
            W2_sb = cp.tile([P, KH, C], bf16, tag="W2")
            nc.sync.dma_start(out=W2_sb[:], in_=t_W2[:])
            h0s_sb = cp.tile([P, NT, H], bf16, tag="h0s")

            # ---- phase 0: h0 = relu(x@W1 + b1); h0s = bf16(0.1*h0);
            #      exch = fp8(dinv*h0) -> AllGather -> tables[0]
            for t in range(NT):
                xt = xp_.tile([P, KF, P], bf16, tag="xt")
                nc.sync.dma_start(out=xt[:], in_=t_xp[t])
                ps = pg.tile([P, H], f32, space="PSUM", tag="gemm")
                for k in range(KF):
                    for nh in range(2):
                        nc.tensor.matmul(
                            out=ps[:, nh * 512:(nh + 1) * 512],
                            lhsT=xt[:, k, :],
                            rhs=W1_sb[:, k, nh * 512:(nh + 1) * 512],
                            start=(k == 0), stop=(k == KF - 1))
                nc.vector.tensor_add(out=ps[:], in0=ps[:], in1=b1_sb[:])
                nc.scalar.activation(out=h0s_sb[:, t, :], in_=ps[:],
                                     func=ACT.Relu, scale=0.1)
                e0 = zp.tile([P, H], fp8, tag="ex")
                nc.scalar.activation(out=e0[:], in_=ps[:], func=ACT.Relu,
                                     scale=dinv_sb[:, t:t + 1])
                nc.sync.dma_start(out=exch[t * P:(t + 1) * P, :], in_=e0[:])
                if t == NT - 7:
                    allgather_half(tables[0], 0)
            allgather_half(tables[0], 1)

            # ---- layers
            qctr = [0]
            for l in range(L):
                tbl = tables[l % 2]
                beta = float(BETAS[l])
                cprime = beta / ((1.0 - beta) * scales[l])
                Wq_sb = wp.tile([P, KH, H], fp8, tag="W")
                nc.sync.dma_start(out=Wq_sb[:], in_=t_Wq[l])
                for t in range(NT):
                    nch_t = int(nch0[t])
                    b8 = int(base0[t]) * 8
                    bS = int(base0[t])
                    g = gp.tile([P, NCHMAX, H], fp8, tag="g")
                    for cc0 in range(0, nch_t, 8):
                        w8 = min(8, nch_t - cc0)
                        nc.gpsimd.dma_gather(
                            out_ap=g[:, cc0:cc0 + w8, :], in_ap=tbl.ap(),
                            idxs_ap=idx_sb[:, b8 + cc0 * 8:
                                           b8 + (cc0 + w8) * 8],
                            num_idxs=w8 * P, num_idxs_reg=w8 * P,
                            elem_size=H, queue_num=qctr[0] % 4)
                        qctr[0] += 1
                    agg = pa.tile([P, H], f32, space="PSUM", tag="agg")
                    for kp in range(nch_t // 2):
                        for nh in range(2):
                            nc.tensor.matmul(
                                out=agg[:, nh * 512:(nh + 1) * 512],
                                lhsT=S_sb[:, bS + 2 * kp:bS + 2 * kp + 2, :],
                                rhs=g[:, 2 * kp:2 * kp + 2,
                                      nh * 512:(nh + 1) * 512],
                                start=(kp == 0), stop=(kp == nch_t // 2 - 1),
                                perf_mode=DR)
                    # z = 0.9*dinv*agg + 0.1*h0
                    z = zp.tile([P, H], f32, tag="z")
                    nc.scalar.activation(out=z[:], in_=agg[:], func=ACT.Copy,
                                         scale=d09_sb[:, t:t + 1])
                    nc.vector.tensor_add(out=z[:], in0=z[:],
                                         in1=h0s_sb[:, t, :])
                    zqT = zp.tile([P, KH, P], fp8, tag="zqT")
                    for k in range(KH):
                        trp = pt.tile([P, P], f32, space="PSUM", tag="tr")
                        nc.tensor.transpose(out=trp[:],
                                            in_=z[:, k * P:(k + 1) * P],
                                            identity=ident[:])
                        nc.vector.tensor_copy(out=zqT[:, k, :], in_=trp[:])
                    ps2 = pg.tile([P, H], f32, space="PSUM", tag="gemm")
                    for kp in range(KH // 2):
                        for nh in range(2):
                            nc.tensor.matmul(
                                out=ps2[:, nh * 512:(nh + 1) * 512],
                                lhsT=zqT[:, 2 * kp:2 * kp + 2, :],
                                rhs=Wq_sb[:, 2 * kp:2 * kp + 2,
                                          nh * 512:(nh + 1) * 512],
                                start=(kp == 0), stop=(kp == KH // 2 - 1),
                                perf_mode=DR)
                    # w = z + (beta/((1-beta)*s))*ps2 ; in-place onto t3
                    w = zp.tile([P, H], f32, tag="w")
                    nc.scalar.activation(out=w[:], in_=ps2[:], func=ACT.Copy,
                                         scale=cprime)
                    nc.vector.tensor_add(out=w[:], in0=w[:], in1=z[:])
                    if l < L - 1:
                        e = zp.tile([P, H], fp8, tag="ex")
                        nc.scalar.activation(
                            out=e[:], in_=w[:], func=ACT.Relu,
                            scale=esc_sb[:, l * NT + t:l * NT + t + 1])
                        nc.sync.dma_start(out=exch[t * P:(t + 1) * P, :],
                                          in_=e[:])
                        if t == NT - 7:
                            allgather_half(tables[(l + 1) % 2], 0)
                    else:
                        h8 = zp.tile([P, H], f32, tag="h8")
                        nc.scalar.activation(out=h8[:], in_=w[:],
                                             func=ACT.Relu,
                                             scale=1.0 - beta)
                        h8T = zp.tile([P, KH, P], bf16, tag="h8T")
                        for k in range(KH):
                            trp = pt.tile([P, P], f32, space="PSUM", tag="tr")
                            nc.tensor.transpose(out=trp[:],
                                                in_=h8[:, k * P:(k + 1) * P],
                                                identity=ident[:])
                            nc.vector.tensor_copy(out=h8T[:, k, :], in_=trp[:])
                        psl = pt.tile([P, P], f32, space="PSUM", tag="tr")
                        for k in range(KH):
                            nc.tensor.matmul(
                                out=psl[:, 0:C],
                                lhsT=h8T[:, k, :],
                                rhs=W2_sb[:, k, :],
                                start=(k == 0), stop=(k == KH - 1))
                        nc.vector.tensor_add(out=psl[:, 0:C], in0=psl[:, 0:C],
                                             in1=b2_sb[:])
                        mx = zp.tile([P, 1], f32, tag="mx")
                        nc.vector.tensor_reduce(out=mx[:], in_=psl[:, 0:C],
                                                axis=mybir.AxisListType.X,
                                                op=mybir.AluOpType.max)
                        nmx = zp.tile([P, 1], f32, tag="nmx")
                        nc.vector.tensor_scalar(
                            out=nmx[:], in0=mx[:], scalar1=-1.0, scalar2=None,
                            op0=mybir.AluOpType.mult)
                        esb = zp.tile([P, C], f32, tag="esb")
                        se = zp.tile([P, 1], f32, tag="se")
                        nc.scalar.activation(out=esb[:], in_=psl[:, 0:C],
                                             func=ACT.Exp,
                                             bias=nmx[:], accum_out=se[:])
                        lse = zp.tile([P, 1], f32, tag="lse")
                        nc.scalar.activation(out=lse[:], in_=se[:],
                                             func=ACT.Ln)
                        o_t = zp.tile([P, C], f32, tag="ot")
                        nc.vector.tensor_scalar(
                            out=o_t[:], in0=psl[:, 0:C], scalar1=mx[:],
                            scalar2=lse[:],
                            op0=mybir.AluOpType.subtract,
                            op1=mybir.AluOpType.subtract)
                        nc.sync.dma_start(out=t_out[t * P:(t + 1) * P, :],
                                          in_=o_t[:])
                if l < L - 1:
                    allgather_half(tables[(l + 1) % 2], 1)
    nc.compile()
    return nc


def kernel(**inputs):
    in_maps, meta = _preprocess(
        inputs["x"], inputs["edge_index"], inputs["W1"], inputs["b1"],
        inputs["Wg"], inputs["W2"], inputs["b2"])
    key = ("nc", meta["TOTC"], tuple(meta["nch"]), tuple(meta["scales"]))
    if key not in _cache:
        _cache[key] = _build(meta)
    nc = _cache[key]
    res = run_bass_kernel_spmd(nc, in_maps, list(range(NCORES)))
    out = np.concatenate(
        [res.results[c]["out"][:SH] for c in range(NCORES)], axis=0)
    return out.astype(np.float32)
